# revision 3
# baseline (speedup 1.0000x reference)
"""Trainium2 Bass kernel for nn_ExBimamba: bidirectional Mamba block.

Sharding: 8 NeuronCores = 4 samples x 2 directions (fwd/bwd). Each core runs one
full Mamba pass for one (sample, direction) plus the final output projection
folded into W_out (host precomputes Wcomb = W_out^T @ Wo_half^T); the host sums
the two partial projections per sample and adds bo.

Per-core layout: channels on partitions, time on free dim.
- depthwise causal conv as 4 accumulating diagonal-stationary matmuls on PE
- softplus batched: all 8 blocks' Exp, then all Ln in-place (3 act-table loads
  total instead of 28)
- dA_n = Exp(A[d,n] * delta) via ACT with per-partition scale pointer
- selective scan via the DVE tensor_tensor_scan instruction (DVE-only opcode),
  two zero-pad-separated (channel-block, n) state segments per instruction
- dBu and C*h elementwise multiplies split between DVE (2x bf16) and the
  otherwise-idle GPSIMD/Pool engine to balance engine occupancy
- B/C rows broadcast across partitions with 0-partition-stride DMA (DRAM bounce)
- y = sum_n C*h via identity-stationary accumulating matmuls (PE), with the
  + xh*D skip connection folded in as a diagonal-stationary matmul
"""
import sys
import os

for _p in ('/opt/trn_rl_repo', os.path.join(os.path.dirname(os.path.abspath(__file__)))):
    if _p not in sys.path:
        sys.path.insert(0, _p)

import numpy as np
import ml_dtypes
from contextlib import ExitStack

import concourse.bass as bass
import concourse.bacc as bacc
import concourse.tile as tile
from concourse import mybir
from concourse.bass_utils import run_bass_kernel_spmd

F32 = mybir.dt.float32
BF16 = mybir.dt.bfloat16
AF = mybir.ActivationFunctionType
OP = mybir.AluOpType

B = 4
L = 1024
D_MODEL = 512
D_IN = 1024
N = 16
DT_RANK = 32
K_CONV = 4

# fraction (num/den) of the 128 big elementwise multiplies routed to Pool
POOL_NUM = 54
POOL_DEN = 128


def _in_shapes():
    return {
        "xT": ((D_MODEL, L + 4), BF16),
        "w1x": ((D_MODEL, D_IN), BF16),
        "w1z": ((D_MODEL, D_IN), BF16),
        "wx": ((D_IN, 2 * N + DT_RANK), BF16),
        "wdt": ((DT_RANK, D_IN), BF16),
        "wcomb": ((D_IN, D_MODEL), BF16),
        "consts": ((D_IN, N + 2), F32),
        "ident": ((128, 128), BF16),
        "ddiag": ((D_IN, 128), BF16),
        "dconv": ((D_IN, K_CONV * 128), BF16),
    }


def _kernel_body(tc, out, ins):
    nc = tc.nc
    SEGL = L + 2
    SPI = 2
    NB = D_IN // 128
    NM = D_MODEL // 128
    TS = 512
    TH = L // TS
    NQ = N // SPI

    with ExitStack() as ctx:
        wpool = ctx.enter_context(tc.tile_pool(name="w", bufs=1))
        pers = ctx.enter_context(tc.tile_pool(name="pers", bufs=1))
        work = ctx.enter_context(tc.tile_pool(name="work", bufs=2))
        spool = ctx.enter_context(tc.tile_pool(name="scan", bufs=2))
        ppool = ctx.enter_context(tc.tile_pool(name="ps", bufs=2, space="PSUM"))
        ypool = ctx.enter_context(tc.tile_pool(name="yps", bufs=1, space="PSUM"))

        def load_rows(name, nchunks, width, dt=BF16, eng=None):
            src = ins[name]
            ts = []
            for c in range(nchunks):
                t = wpool.tile([128, width], dt, tag=f"{name}{c}", name=f"{name}{c}")
                (eng or nc.sync).dma_start(t[:], src[c * 128:(c + 1) * 128, :])
                ts.append(t)
            return ts

        # critical-path loads on the SP queue, in need-order; the rest later
        xT_sb = load_rows("xT", NM, L + 4)
        w1x_sb = load_rows("w1x", NM, D_IN)
        cst_sb = load_rows("consts", NB, N + 2, F32)
        wx_sb = load_rows("wx", NB, 2 * N + DT_RANK)
        w1z_sb = load_rows("w1z", NM, D_IN)
        wc_sb = load_rows("wcomb", NB, D_MODEL)
        A_sb = cst_sb
        cb_sb = [t[:, N:N + 1] for t in cst_sb]
        bdt_sb = [t[:, N + 1:N + 2] for t in cst_sb]
        wdt_sb = wpool.tile([DT_RANK, D_IN], BF16)
        nc.sync.dma_start(wdt_sb[:], ins["wdt"][:, :])
        id_sb = wpool.tile([128, 128], BF16)
        nc.sync.dma_start(id_sb[:], ins["ident"][:, :])

        # phase B: xh = silu(depthwise_conv(W1x x) + conv_b)
        # conv as 4 accumulating diagonal-stationary matmuls on PE
        zs_dram = nc.dram_tensor("zs_scratch", [D_IN, L], BF16, kind="Internal").ap()
        xh_sb = [pers.tile([128, L], BF16, tag=f"xh{b}", name=f"xh{b}") for b in range(NB)]
        for b in range(NB):
            xpre = work.tile([128, L + 3], BF16, tag="xpre")
            nc.vector.memset(xpre[:, 0:3], 0.0)
            for th in range(TH):
                ps = ppool.tile([128, TS], F32, tag="pB")
                for cm in range(NM):
                    nc.tensor.matmul(
                        ps[:], w1x_sb[cm][:, b * 128:(b + 1) * 128],
                        xT_sb[cm][:, 3 + th * TS: 3 + th * TS + TS],
                        start=(cm == 0), stop=(cm == NM - 1))
                nc.scalar.copy(xpre[:, 3 + th * TS: 3 + (th + 1) * TS], ps[:])
            dcv = work.tile([128, K_CONV * 128], BF16, tag="dcv", name=f"dcv{b}")
            nc.scalar.dma_start(dcv[:], ins["dconv"][b * 128:(b + 1) * 128, :])
            for th in range(TH):
                cps = ppool.tile([128, TS], F32, tag="pp")
                for k in range(K_CONV):
                    nc.tensor.matmul(
                        cps[:], dcv[:, k * 128:(k + 1) * 128],
                        xpre[:, k + th * TS: k + th * TS + TS],
                        start=(k == 0), stop=(k == K_CONV - 1))
                nc.scalar.activation(xh_sb[b][:, th * TS:(th + 1) * TS], cps[:],
                                     AF.Silu, bias=cb_sb[b])

        # phase C: x_dbl = xh @ Wx^T
        dt_sb = pers.tile([DT_RANK, L], BF16)
        bc_sb = pers.tile([2 * N, L], BF16)
        for th in range(TH):
            ps = ppool.tile([2 * N + DT_RANK, TS], F32, tag="pp")
            for b in range(NB):
                nc.tensor.matmul(ps[:], wx_sb[b][:, :], xh_sb[b][:, th * TS:(th + 1) * TS],
                                 start=(b == 0), stop=(b == NB - 1))
            nc.scalar.copy(dt_sb[:, th * TS:(th + 1) * TS], ps[0:DT_RANK, :])
            nc.scalar.copy(bc_sb[:, th * TS:(th + 1) * TS], ps[DT_RANK:2 * N + DT_RANK, :])

        # phase D: broadcast B,C rows across partitions (DRAM bounce, 0-stride read)
        bc_dram = nc.dram_tensor("bc_scratch", [2 * N, L], BF16, kind="Internal").ap()
        nc.sync.dma_start(bc_dram[:, :], bc_sb[:])
        Bbig = pers.tile([128, N * L], BF16)
        Cbig = pers.tile([128, N * L], BF16)
        for n in range(N):
            for big, row in ((Bbig, n), (Cbig, N + n)):
                src = bc_dram[row:row + 1, :]
                src_b = bass.AP(tensor=src.tensor, offset=src.offset,
                                ap=[[0, 128]] + [list(d) for d in src.ap[1:]])
                nc.sync.dma_start(big[:, n * L: (n + 1) * L], src_b)

        # phase B2: z-gate matmuls (emitted after C/D so they don't delay the
        # critical path; PE fills its slack during early phase E)
        for b in range(NB):
            zt = work.tile([128, L], BF16, tag="zt", bufs=1)
            for th in range(TH):
                psz = ppool.tile([128, TS], F32, tag="pB")
                for cm in range(NM):
                    nc.tensor.matmul(
                        psz[:], w1z_sb[cm][:, b * 128:(b + 1) * 128],
                        xT_sb[cm][:, 3 + th * TS: 3 + th * TS + TS],
                        start=(cm == 0), stop=(cm == NM - 1))
                nc.scalar.activation(zt[:, th * TS:(th + 1) * TS], psz[:], AF.Silu)
            nc.sync.dma_start(zs_dram[b * 128:(b + 1) * 128, :], zt[:])

        # phase SP: softplus for all blocks, batched per activation function so
        # the ACT table is loaded at most 3 times total (Silu/Exp/Ln)
        delta_sb = [pers.tile([128, L], BF16, tag=f"dl{b}", name=f"dl{b}")
                    for b in range(NB)]
        for b in range(NB):
            zpre = ppool.tile([128, L], F32, tag="zpre", bufs=1)
            for th in range(TH):
                nc.tensor.matmul(zpre[:, th * TS:(th + 1) * TS],
                                 wdt_sb[:, b * 128:(b + 1) * 128],
                                 dt_sb[:, th * TS:(th + 1) * TS],
                                 start=True, stop=True)
            nc.scalar.activation(delta_sb[b][:], zpre[:], AF.Exp, bias=bdt_sb[b])
        for b in range(NB):
            # in-place: delta = ln(exp(zpre+bdt) + 1) = softplus(zpre + bdt)
            nc.scalar.activation(delta_sb[b][:], delta_sb[b][:], AF.Ln, bias=1.0)

        # phase E: per channel-block: u, dA, scan, y
        y4_sb = [pers.tile([128, L], BF16, tag=f"y4{b}", name=f"y4{b}") for b in range(NB)]
        d0_pp = [spool.tile([128, SPI * SEGL], BF16, tag=f"d0{i}", bufs=1, name=f"d0pp{i}")
                 for i in range(2)]
        d1_pp = [spool.tile([128, SPI * SEGL], BF16, tag=f"d1{i}", bufs=1, name=f"d1pp{i}")
                 for i in range(2)]
        h_pp = [spool.tile([128, SPI * SEGL], BF16, tag=f"h{i}", bufs=1, name=f"hpp{i}")
                for i in range(2)]
        for dd in d0_pp + d1_pp:
            pad = bass.AP(tensor=dd.tensor, offset=dd.offset + L,
                          ap=[list(dd.ap[0]), [SEGL, SPI], [1, SEGL - L]])
            nc.vector.memset(pad, 0.0)

        # Bresenham split of the 128 big multiplies between Pool and DVE
        tt_state = [0]

        def tt_engine():
            tt_state[0] += POOL_NUM
            if tt_state[0] >= POOL_DEN:
                tt_state[0] -= POOL_DEN
                return nc.gpsimd
            return nc.vector

        for b in range(NB):
            u = work.tile([128, L], BF16, tag="u", bufs=1)
            nc.vector.tensor_mul(u[:], delta_sb[b][:], xh_sb[b][:])

            yps = ypool.tile([128, L], F32, tag="yps")
            for q in range(NQ):
                d0 = d0_pp[q % 2]
                d1 = d1_pp[q % 2]
                for nn in range(SPI):
                    n = q * SPI + nn
                    nc.scalar.activation(d0[:, nn * SEGL: nn * SEGL + L], delta_sb[b][:],
                                         AF.Exp, scale=A_sb[b][:, n:n + 1])
                # one fused multiply for both segments: u re-read via 0-stride dim
                d1_out = bass.AP(tensor=d1.tensor, offset=d1.offset,
                                 ap=[list(d1.ap[0]), [SEGL, SPI], [1, L]])
                u_b = bass.AP(tensor=u.tensor, offset=u.offset,
                              ap=[list(u.ap[0]), [0, SPI], [1, L]])
                bslc = Bbig[:, q * SPI * L: (q + 1) * SPI * L]
                b_in = bass.AP(tensor=bslc.tensor, offset=bslc.offset,
                               ap=[list(bslc.ap[0]), [L, SPI], [1, L]])
                tt_engine().tensor_tensor(d1_out, u_b, b_in, OP.mult)
                h = h_pp[q % 2]
                nc.vector.tensor_tensor_scan(h[:], d0[:], d1[:], 0.0, OP.mult, OP.add)
                p = spool.tile([128, SPI * L], BF16, tag="p", bufs=2)
                h_in = bass.AP(tensor=h.tensor, offset=h.offset,
                               ap=[list(h.ap[0]), [SEGL, SPI], [1, L]])
                tt_engine().tensor_tensor(p[:], h_in, Cbig[:, q * SPI * L:(q + 1) * SPI * L],
                                          OP.mult)
                for nn in range(SPI):
                    n = q * SPI + nn
                    for th in range(TH):
                        nc.tensor.matmul(
                            yps[:, th * TS:(th + 1) * TS], id_sb[:],
                            p[:, nn * L + th * TS: nn * L + th * TS + TS],
                            start=(n == 0 and th in (0, 1)), stop=False)
            dd = wpool.tile([128, 128], BF16, tag="ddiag", bufs=2, name=f"dd{b}")
            nc.sync.dma_start(dd[:], ins["ddiag"][b * 128:(b + 1) * 128, :])
            for th in range(TH):
                nc.tensor.matmul(yps[:, th * TS:(th + 1) * TS], dd[:],
                                 xh_sb[b][:, th * TS:(th + 1) * TS],
                                 start=False, stop=True)
            zs = work.tile([128, L], BF16, tag="zs", bufs=1)
            nc.sync.dma_start(zs[:], zs_dram[b * 128:(b + 1) * 128, :])
            ysb = work.tile([128, L], BF16, tag="ysb", bufs=1)
            nc.scalar.copy(ysb[:], yps[:])
            nc.vector.tensor_mul(y4_sb[b][:], ysb[:], zs[:])

        # phase F: partial final output = y4 @ Wcomb^T (Wcomb = W_out^T Wo_half^T
        # folded on the host, eliminating the separate Wo projection)
        for jo in range(NM):
            o_sb = work.tile([128, L], F32, tag="osb", bufs=1)
            for th in range(TH):
                ps = ppool.tile([128, TS], F32, tag="pp")
                for b in range(NB):
                    nc.tensor.matmul(ps[:], wc_sb[b][:, jo * 128:(jo + 1) * 128],
                                     y4_sb[b][:, th * TS:(th + 1) * TS],
                                     start=(b == 0), stop=(b == NB - 1))
                if th == 0:
                    nc.vector.tensor_copy(o_sb[:, th * TS:(th + 1) * TS], ps[:])
                else:
                    nc.scalar.copy(o_sb[:, th * TS:(th + 1) * TS], ps[:])
            eng = nc.sync if jo % 2 == 0 else nc.scalar
            eng.dma_start(out[jo * 128:(jo + 1) * 128, :], o_sb[:])


_NC_CACHE = None


def _build_nc():
    global _NC_CACHE
    if _NC_CACHE is not None:
        return _NC_CACHE
    nc = bacc.Bacc("TRN2", target_bir_lowering=False, debug=False, num_devices=8)
    ins = {}
    for name, (shape, dt) in _in_shapes().items():
        ins[name] = nc.dram_tensor(name, list(shape), dt, kind="ExternalInput").ap()
    out = nc.dram_tensor("out", [D_MODEL, L], F32, kind="ExternalOutput").ap()
    with tile.TileContext(nc) as tc:
        _kernel_body(tc, out, ins)
    nc.compile()
    _NC_CACHE = nc
    return nc


def _prep_core_inputs(x, p):
    """x: (L, 512) f32 input for this core; p: dict with this direction's params
    plus 'wo_half' (512, 512) = Wo[:, half].T."""
    bf = ml_dtypes.bfloat16
    xTp = np.zeros((D_MODEL, L + 4), np.float32)
    xTp[:, 3:3 + L] = x.T
    W_in = p['W_in']
    conv_w = p['conv_w'][:, 0, :]
    consts = np.concatenate([
        -np.exp(p['A_log']).astype(np.float32),
        p['conv_b'].reshape(-1, 1).astype(np.float32),
        p['b_dt'].reshape(-1, 1).astype(np.float32)], axis=1)
    # fold the two output projections: out = Wo_half @ (W_out @ y4) = Wcomb^T y4
    wcomb = (p['W_out'].astype(np.float64).T @ p['wo_half'].astype(np.float64))
    dconv = np.zeros((D_IN, K_CONV * 128), np.float32)
    for b in range(D_IN // 128):
        for k in range(K_CONV):
            blk = dconv[b * 128:(b + 1) * 128, k * 128:(k + 1) * 128]
            np.fill_diagonal(blk, conv_w[b * 128:(b + 1) * 128, k])
    return {
        "xT": xTp.astype(bf),
        "w1x": np.ascontiguousarray(W_in[:D_IN, :].T).astype(bf),
        "w1z": np.ascontiguousarray(W_in[D_IN:, :].T).astype(bf),
        "wx": np.ascontiguousarray(p['W_x'].T).astype(bf),
        "wdt": np.ascontiguousarray(p['W_dt'].T).astype(bf),
        "wcomb": np.ascontiguousarray(wcomb).astype(np.float32).astype(bf),
        "consts": np.ascontiguousarray(consts).astype(np.float32),
        "ident": np.eye(128, dtype=bf),
        "ddiag": np.concatenate([np.diag(p['D'][b * 128:(b + 1) * 128])
                                 for b in range(D_IN // 128)], axis=0).astype(bf),
        "dconv": dconv.astype(bf),
    }


def _dir_params(inputs, prefix, wo_half):
    names = ['W_in', 'conv_w', 'conv_b', 'W_x', 'W_dt', 'b_dt', 'A_log', 'D', 'W_out']
    p = {n: np.asarray(inputs[prefix + n], np.float32) for n in names}
    p['wo_half'] = wo_half
    return p


def _masked_flip(x, lengths):
    L_ = x.shape[1]
    j = np.arange(L_)[None, :]
    idx = np.where(j < lengths[:, None], lengths[:, None] - 1 - j, j)
    return np.take_along_axis(x, idx[:, :, None], axis=1)


def kernel(**inputs):
    nc = _build_nc()
    hidden = np.asarray(inputs['hidden_input'], np.float32)   # (B, L, 512)
    mask = np.asarray(inputs['mask'], np.int32)
    Wo = np.asarray(inputs['Wo'], np.float32)                 # (512, 1024)
    bo = np.asarray(inputs['bo'], np.float32)

    lengths = mask.sum(axis=1)
    bwd_in = _masked_flip(hidden, lengths)

    pf = _dir_params(inputs, 'f_', np.ascontiguousarray(Wo[:, :D_MODEL].T))
    pb = _dir_params(inputs, 'b_', np.ascontiguousarray(Wo[:, D_MODEL:].T))

    in_maps = []
    for i in range(B):
        in_maps.append(_prep_core_inputs(hidden[i], pf))
    for i in range(B):
        in_maps.append(_prep_core_inputs(bwd_in[i], pb))

    res = run_bass_kernel_spmd(nc, in_maps, core_ids=list(range(8)))

    out = np.empty((B, L, D_MODEL), np.float32)
    for i in range(B):
        fwd = res.results[i]["out"].T                       # (L, 512)
        bwd_f = res.results[B + i]["out"].T                 # (L, 512), flipped time
        bwd = _masked_flip(bwd_f[None], lengths[i:i + 1])[0]
        out[i] = fwd + bwd + bo
    return out


# revision 11
# speedup vs baseline: 1.0187x; 1.0187x over previous
"""Trainium2 Bass kernel for nn_ExBimamba: bidirectional Mamba block.

Sharding: 8 NeuronCores = 4 samples x 2 directions (fwd/bwd). Each core runs one
full Mamba pass for one (sample, direction) plus the final output projection
folded into W_out (host precomputes Wcomb = W_out^T @ Wo_half^T); the host sums
the two partial projections per sample and adds bo.

Per-core layout: channels on partitions, time on free dim.
- depthwise causal conv on DVE (tensor-scalar 4x mode) during the PE-bound
  input-matmul lead-in when DVE is otherwise idle
- x_dbl accumulation matmuls interleaved into the input phase per block
- softplus batched: all 8 blocks' Exp, then all Ln in-place, then z-gate silus,
  so the ACT function table is swapped only a handful of times
- dA_n = Exp(A[d,n] * delta) via ACT with per-partition scale pointer
- selective scan via the DVE tensor_tensor_scan instruction (DVE-only opcode),
  two zero-pad-separated (channel-block, n) state segments per instruction
- dBu and C*h elementwise multiplies split between DVE (tensor_tensor, 2x bf16)
  and the otherwise-idle GPSIMD/Pool engine (scalar_tensor_tensor, which maps
  to the faster 0.6-efficiency path) to balance engine occupancy
- B/C rows broadcast across partitions with 0-partition-stride DMA (DRAM bounce)
- y = sum_n C*h via identity-stationary accumulating matmuls (PE), with the
  + xh*D skip connection folded in as a diagonal-stationary matmul
"""
import sys
import os

for _p in ('/opt/trn_rl_repo', os.path.join(os.path.dirname(os.path.abspath(__file__)))):
    if _p not in sys.path:
        sys.path.insert(0, _p)

import numpy as np
import ml_dtypes
from contextlib import ExitStack

import concourse.bass as bass
import concourse.bacc as bacc
import concourse.tile as tile
from concourse import mybir
from concourse.bass_utils import run_bass_kernel_spmd

F32 = mybir.dt.float32
BF16 = mybir.dt.bfloat16
AF = mybir.ActivationFunctionType
OP = mybir.AluOpType

B = 4
L = 1024
D_MODEL = 512
D_IN = 1024
N = 16
DT_RANK = 32
K_CONV = 4

# fraction (num/den) of the 128 big elementwise multiplies routed to Pool
POOL_NUM = 54
POOL_DEN = 128


def _in_shapes():
    return {
        "xT": ((D_MODEL, L + 4), BF16),
        "w1x": ((D_MODEL, D_IN), BF16),
        "w1z": ((D_MODEL, D_IN), BF16),
        "wx": ((D_IN, 2 * N + DT_RANK), BF16),
        "wdt": ((DT_RANK, D_IN), BF16),
        "wcomb": ((D_IN, D_MODEL), BF16),
        "consts": ((D_IN, N + 3 + K_CONV), F32),
        "ident": ((128, 128), BF16),
        "ddiag": ((D_IN, 128), BF16),
    }


def _kernel_body(tc, out, ins):
    nc = tc.nc
    SEGL = L + 2
    SPI = 2
    NB = D_IN // 128
    NM = D_MODEL // 128
    TS = 512
    TH = L // TS
    NQ = N // SPI

    with ExitStack() as ctx:
        wpool = ctx.enter_context(tc.tile_pool(name="w", bufs=1))
        pers = ctx.enter_context(tc.tile_pool(name="pers", bufs=1))
        work = ctx.enter_context(tc.tile_pool(name="work", bufs=2))
        spool = ctx.enter_context(tc.tile_pool(name="scan", bufs=2))
        ppool = ctx.enter_context(tc.tile_pool(name="ps", bufs=2, space="PSUM"))
        ypool = ctx.enter_context(tc.tile_pool(name="yps", bufs=1, space="PSUM"))

        def load_rows(name, nchunks, width, dt=BF16, eng=None):
            src = ins[name]
            ts = []
            for c in range(nchunks):
                t = wpool.tile([128, width], dt, tag=f"{name}{c}", name=f"{name}{c}")
                (eng or nc.sync).dma_start(t[:], src[c * 128:(c + 1) * 128, :])
                ts.append(t)
            return ts

        # critical-path loads on the SP queue, in need-order
        xT_sb = load_rows("xT", NM, L + 4)
        w1x_sb = load_rows("w1x", NM, D_IN)
        cst_sb = load_rows("consts", NB, N + 3 + K_CONV, F32)
        wx_sb = load_rows("wx", NB, 2 * N + DT_RANK)
        w1z_sb = load_rows("w1z", NM, D_IN)
        wc_sb = load_rows("wcomb", NB, D_MODEL)
        A_sb = cst_sb
        cb_sb = [t[:, N:N + 1] for t in cst_sb]
        bdt_sb = [t[:, N + 1:N + 2] for t in cst_sb]
        cw_sb = [[t[:, N + 3 + k:N + 4 + k] for k in range(K_CONV)] for t in cst_sb]
        wdt_sb = wpool.tile([DT_RANK, D_IN], BF16)
        nc.sync.dma_start(wdt_sb[:], ins["wdt"][:, :])
        id_sb = wpool.tile([128, 128], BF16)
        nc.sync.dma_start(id_sb[:], ins["ident"][:, :])

        # phase B: xh matmul -> xpre; depthwise conv on DVE (idle in lead-in);
        # silu; x_dbl accumulation matmuls folded in per block
        zs_dram = nc.dram_tensor("zs_scratch", [D_IN, L], BF16, kind="Internal").ap()
        xh_sb = [pers.tile([128, L], BF16, tag=f"xh{b}", name=f"xh{b}") for b in range(NB)]
        # dt/bc die after phase SP/D; their slots are reused by zt (B2) and
        # osb (F) via shared tags
        dt_sb = pers.tile([DT_RANK, L], BF16, tag="dtz")
        bc_sb = work.tile([2 * N, L], BF16, tag="osb")
        ppc = [ppool.tile([2 * N + DT_RANK, TS], F32, tag=f"ppc{th}", bufs=1,
                          name=f"ppc{th}") for th in range(TH)]
        for b in range(NB):
            xpre = work.tile([128, L + 3], BF16, tag="xpre")
            nc.vector.memset(xpre[:, 0:3], 0.0)
            for th in range(TH):
                ps = ppool.tile([128, TS], F32, tag="pB", bufs=4)
                for cm in range(NM):
                    nc.tensor.matmul(
                        ps[:], w1x_sb[cm][:, b * 128:(b + 1) * 128],
                        xT_sb[cm][:, 3 + th * TS: 3 + th * TS + TS],
                        start=(cm == 0), stop=(cm == NM - 1))
                nc.scalar.copy(xpre[:, 3 + th * TS: 3 + (th + 1) * TS], ps[:])
            t0 = work.tile([128, L], BF16, tag="ct0", bufs=1, name=f"ct0_{b}")
            t1 = work.tile([128, L], BF16, tag="ct1", bufs=1, name=f"ct1_{b}")
            nc.vector.tensor_scalar_mul(t0[:], xpre[:, 0:L], cw_sb[b][0])
            for k in range(1, K_CONV):
                nc.vector.tensor_scalar_mul(t1[:], xpre[:, k:k + L], cw_sb[b][k])
                nc.vector.tensor_add(t0[:], t0[:], t1[:])
            nc.scalar.activation(xh_sb[b][:], t0[:], AF.Silu, bias=cb_sb[b])
            # x_dbl partial accumulation for this block
            for th in range(TH):
                nc.tensor.matmul(ppc[th][:], wx_sb[b][:, :],
                                 xh_sb[b][:, th * TS:(th + 1) * TS],
                                 start=(b == 0), stop=(b == NB - 1))
        for th in range(TH):
            nc.scalar.copy(dt_sb[:, th * TS:(th + 1) * TS], ppc[th][0:DT_RANK, :])
            nc.scalar.copy(bc_sb[:, th * TS:(th + 1) * TS],
                           ppc[th][DT_RANK:2 * N + DT_RANK, :])

        # phase D: broadcast B,C rows across partitions (DRAM bounce, 0-stride read)
        bc_dram = nc.dram_tensor("bc_scratch", [2 * N, L], BF16, kind="Internal").ap()
        nc.sync.dma_start(bc_dram[:, :], bc_sb[:])
        Bbig = pers.tile([128, N * L], BF16)
        Cbig = pers.tile([128, N * L], BF16)
        for n in range(N):
            for big, row in ((Bbig, n), (Cbig, N + n)):
                src = bc_dram[row:row + 1, :]
                src_b = bass.AP(tensor=src.tensor, offset=src.offset,
                                ap=[[0, 128]] + [list(d) for d in src.ap[1:]])
                nc.sync.dma_start(big[:, n * L: (n + 1) * L], src_b)

        # phase SP: softplus for all blocks, batched per activation function
        delta_sb = [pers.tile([128, L], BF16, tag=f"dl{b}", name=f"dl{b}")
                    for b in range(NB)]
        for b in range(NB):
            for th in range(TH):
                zpre = ppool.tile([128, TS], F32, tag="pB", bufs=4)
                nc.tensor.matmul(zpre[:], wdt_sb[:, b * 128:(b + 1) * 128],
                                 dt_sb[:, th * TS:(th + 1) * TS],
                                 start=True, stop=True)
                nc.scalar.activation(delta_sb[b][:, th * TS:(th + 1) * TS], zpre[:],
                                     AF.Exp, bias=bdt_sb[b])
        for b in range(NB):
            # in-place: delta = ln(exp(zpre+bdt) + 1) = softplus(zpre + bdt)
            nc.scalar.activation(delta_sb[b][:], delta_sb[b][:], AF.Ln, bias=1.0)

        # phase B2: z-gate matmuls (after SP so they don't delay the softplus
        # path; PE fills its slack during early phase E)
        for b in range(NB):
            zt = pers.tile([128, L], BF16, tag="dtz", name=f"zt{b}")
            for th in range(TH):
                psz = ppool.tile([128, TS], F32, tag="pB", bufs=4)
                for cm in range(NM):
                    nc.tensor.matmul(
                        psz[:], w1z_sb[cm][:, b * 128:(b + 1) * 128],
                        xT_sb[cm][:, 3 + th * TS: 3 + th * TS + TS],
                        start=(cm == 0), stop=(cm == NM - 1))
                nc.scalar.activation(zt[:, th * TS:(th + 1) * TS], psz[:], AF.Silu)
            nc.sync.dma_start(zs_dram[b * 128:(b + 1) * 128, :], zt[:])

        # phase E: per channel-block: u, dA, scan, y
        y4_sb = [pers.tile([128, L], BF16, tag=f"y4{b}", name=f"y4{b}") for b in range(NB)]
        d0_pp = [spool.tile([128, SPI * SEGL], BF16, tag=f"d0{i}", bufs=1, name=f"d0pp{i}")
                 for i in range(3)]
        d1_pp = [spool.tile([128, SPI * SEGL], BF16, tag=f"d1{i}", bufs=1, name=f"d1pp{i}")
                 for i in range(2)]
        h_pp = [spool.tile([128, SPI * SEGL], BF16, tag=f"h{i}", bufs=1, name=f"hpp{i}")
                for i in range(2)]
        for dd in d0_pp + d1_pp:
            pad = bass.AP(tensor=dd.tensor, offset=dd.offset + L,
                          ap=[list(dd.ap[0]), [SEGL, SPI], [1, SEGL - L]])
            nc.vector.memset(pad, 0.0)

        # Bresenham split of the 128 big multiplies between Pool and DVE
        tt_state = [0]

        def tt_mult(out_ap, in0_ap, in1_ap):
            tt_state[0] += POOL_NUM
            if tt_state[0] >= POOL_DEN:
                tt_state[0] -= POOL_DEN
                nc.gpsimd.tensor_tensor(out_ap, in0_ap, in1_ap, OP.mult)
            else:
                nc.vector.tensor_tensor(out_ap, in0_ap, in1_ap, OP.mult)

        for b in range(NB):
            u = work.tile([128, L], BF16, tag="u", bufs=1)
            nc.vector.tensor_mul(u[:], delta_sb[b][:], xh_sb[b][:])

            yps = ypool.tile([128, L], F32, tag="yps")
            for q in range(NQ):
                d0 = d0_pp[q % 3]
                d1 = d1_pp[q % 2]
                for nn in range(SPI):
                    n = q * SPI + nn
                    nc.scalar.activation(d0[:, nn * SEGL: nn * SEGL + L], delta_sb[b][:],
                                         AF.Exp, scale=A_sb[b][:, n:n + 1])
                # one fused multiply for both segments: u re-read via 0-stride dim
                d1_out = bass.AP(tensor=d1.tensor, offset=d1.offset,
                                 ap=[list(d1.ap[0]), [SEGL, SPI], [1, L]])
                u_b = bass.AP(tensor=u.tensor, offset=u.offset,
                              ap=[list(u.ap[0]), [0, SPI], [1, L]])
                bslc = Bbig[:, q * SPI * L: (q + 1) * SPI * L]
                b_in = bass.AP(tensor=bslc.tensor, offset=bslc.offset,
                               ap=[list(bslc.ap[0]), [L, SPI], [1, L]])
                tt_mult(d1_out, u_b, b_in)
                h = h_pp[q % 2]
                nc.vector.tensor_tensor_scan(h[:], d0[:], d1[:], 0.0, OP.mult, OP.add)
                p = spool.tile([128, SPI * L], BF16, tag="p", bufs=2)
                h_in = bass.AP(tensor=h.tensor, offset=h.offset,
                               ap=[list(h.ap[0]), [SEGL, SPI], [1, L]])
                tt_mult(p[:], h_in, Cbig[:, q * SPI * L:(q + 1) * SPI * L])
                for nn in range(SPI):
                    n = q * SPI + nn
                    for th in range(TH):
                        nc.tensor.matmul(
                            yps[:, th * TS:(th + 1) * TS], id_sb[:],
                            p[:, nn * L + th * TS: nn * L + th * TS + TS],
                            start=(n == 0 and th in (0, 1)), stop=False)
            dd = wpool.tile([128, 128], BF16, tag="ddiag", bufs=2, name=f"dd{b}")
            nc.sync.dma_start(dd[:], ins["ddiag"][b * 128:(b + 1) * 128, :])
            for th in range(TH):
                nc.tensor.matmul(yps[:, th * TS:(th + 1) * TS], dd[:],
                                 xh_sb[b][:, th * TS:(th + 1) * TS],
                                 start=False, stop=True)
            zs = work.tile([128, L], BF16, tag="zs", bufs=1)
            nc.sync.dma_start(zs[:], zs_dram[b * 128:(b + 1) * 128, :])
            ysb = work.tile([128, L], BF16, tag="ysb", bufs=1)
            nc.scalar.copy(ysb[:], yps[:])
            nc.vector.tensor_mul(y4_sb[b][:], ysb[:], zs[:])

        # phase F: partial final output = y4 @ Wcomb^T (Wcomb = W_out^T Wo_half^T
        # folded on the host, eliminating the separate Wo projection)
        for jo in range(NM):
            for th in range(TH):
                ps = ppool.tile([128, TS], F32, tag="pB", bufs=4)
                for b in range(NB):
                    nc.tensor.matmul(ps[:], wc_sb[b][:, jo * 128:(jo + 1) * 128],
                                     y4_sb[b][:, th * TS:(th + 1) * TS],
                                     start=(b == 0), stop=(b == NB - 1))
                o_sb = work.tile([128, TS], F32, tag="osb")
                nc.vector.tensor_copy(o_sb[:], ps[:])
                nc.sync.dma_start(out[jo * 128:(jo + 1) * 128,
                                      th * TS:(th + 1) * TS], o_sb[:])


_NC_CACHE = None


def _build_nc():
    global _NC_CACHE
    if _NC_CACHE is not None:
        return _NC_CACHE
    nc = bacc.Bacc("TRN2", target_bir_lowering=False, debug=False, num_devices=8)
    ins = {}
    for name, (shape, dt) in _in_shapes().items():
        ins[name] = nc.dram_tensor(name, list(shape), dt, kind="ExternalInput").ap()
    out = nc.dram_tensor("out", [D_MODEL, L], F32, kind="ExternalOutput").ap()
    with tile.TileContext(nc) as tc:
        _kernel_body(tc, out, ins)
    nc.compile()
    _NC_CACHE = nc
    return nc


def _prep_core_inputs(x, p):
    """x: (L, 512) f32 input for this core; p: dict with this direction's params
    plus 'wo_half' (512, 512) = Wo[:, half].T."""
    bf = ml_dtypes.bfloat16
    xTp = np.zeros((D_MODEL, L + 4), np.float32)
    xTp[:, 3:3 + L] = x.T
    W_in = p['W_in']
    conv_w = p['conv_w'][:, 0, :]
    consts = np.concatenate([
        -np.exp(p['A_log']).astype(np.float32),
        p['conv_b'].reshape(-1, 1).astype(np.float32),
        p['b_dt'].reshape(-1, 1).astype(np.float32),
        p['D'].reshape(-1, 1).astype(np.float32),
        conv_w.astype(np.float32)], axis=1)
    # fold the two output projections: out = Wo_half @ (W_out @ y4) = Wcomb^T y4
    wcomb = (p['W_out'].astype(np.float64).T @ p['wo_half'].astype(np.float64))
    return {
        "xT": xTp.astype(bf),
        "w1x": np.ascontiguousarray(W_in[:D_IN, :].T).astype(bf),
        "w1z": np.ascontiguousarray(W_in[D_IN:, :].T).astype(bf),
        "wx": np.ascontiguousarray(p['W_x'].T).astype(bf),
        "wdt": np.ascontiguousarray(p['W_dt'].T).astype(bf),
        "wcomb": np.ascontiguousarray(wcomb).astype(np.float32).astype(bf),
        "consts": np.ascontiguousarray(consts).astype(np.float32),
        "ident": np.eye(128, dtype=bf),
        "ddiag": np.concatenate([np.diag(p['D'][b * 128:(b + 1) * 128])
                                 for b in range(D_IN // 128)], axis=0).astype(bf),
    }


def _dir_params(inputs, prefix, wo_half):
    names = ['W_in', 'conv_w', 'conv_b', 'W_x', 'W_dt', 'b_dt', 'A_log', 'D', 'W_out']
    p = {n: np.asarray(inputs[prefix + n], np.float32) for n in names}
    p['wo_half'] = wo_half
    return p


def _masked_flip(x, lengths):
    L_ = x.shape[1]
    j = np.arange(L_)[None, :]
    idx = np.where(j < lengths[:, None], lengths[:, None] - 1 - j, j)
    return np.take_along_axis(x, idx[:, :, None], axis=1)


def kernel(**inputs):
    nc = _build_nc()
    hidden = np.asarray(inputs['hidden_input'], np.float32)   # (B, L, 512)
    mask = np.asarray(inputs['mask'], np.int32)
    Wo = np.asarray(inputs['Wo'], np.float32)                 # (512, 1024)
    bo = np.asarray(inputs['bo'], np.float32)

    lengths = mask.sum(axis=1)
    bwd_in = _masked_flip(hidden, lengths)

    pf = _dir_params(inputs, 'f_', np.ascontiguousarray(Wo[:, :D_MODEL].T))
    pb = _dir_params(inputs, 'b_', np.ascontiguousarray(Wo[:, D_MODEL:].T))

    in_maps = []
    for i in range(B):
        in_maps.append(_prep_core_inputs(hidden[i], pf))
    for i in range(B):
        in_maps.append(_prep_core_inputs(bwd_in[i], pb))

    res = run_bass_kernel_spmd(nc, in_maps, core_ids=list(range(8)))

    out = np.empty((B, L, D_MODEL), np.float32)
    for i in range(B):
        fwd = res.results[i]["out"].T                       # (L, 512)
        bwd_f = res.results[B + i]["out"].T                 # (L, 512), flipped time
        bwd = _masked_flip(bwd_f[None], lengths[i:i + 1])[0]
        out[i] = fwd + bwd + bo
    return out


# revision 12
# speedup vs baseline: 1.1189x; 1.0984x over previous
"""Trainium2 Bass kernel for nn_ExBimamba: bidirectional Mamba block.

Sharding: 8 NeuronCores = 4 samples x 2 directions (fwd/bwd). Each core runs one
full Mamba pass for one (sample, direction) plus the final output projection
folded into W_out (host precomputes Wcomb = W_out^T @ Wo_half^T); the host sums
the two partial projections per sample and adds bo.

Per-core layout: channels on partitions, time on free dim.
- phase B per block: input matmul, depthwise conv (DVE tensor-scalar tree while
  PE is busy), silu, z-gate matmul + silu, x_dbl accumulation -- one Silu table
  residency for the whole phase, PE kept continuously warm
- softplus batched: all 8 blocks' Exp, then all Ln in-place (few table loads)
- dA_n = Exp(A[d,n] * delta) via ACT with per-partition scale pointer
- selective scan via the DVE tensor_tensor_scan instruction (DVE-only opcode),
  two zero-pad-separated (channel-block, n) state segments per instruction
- dBu on DVE (feeds the scan on the same queue); C*h split between the
  otherwise-idle GPSIMD/Pool engine and DVE to balance occupancy
- B/C rows broadcast across partitions with 0-partition-stride DMA (DRAM
  bounce); B_n tiles recycle the SBUF slots of xT/w1x/w1z, which are dead
  after phase B
- y = sum_n C*h via identity-stationary accumulating matmuls (PE), with the
  + xh*D skip connection folded in as a diagonal-stationary matmul
"""
import sys
import os

for _p in ('/opt/trn_rl_repo', os.path.join(os.path.dirname(os.path.abspath(__file__)))):
    if _p not in sys.path:
        sys.path.insert(0, _p)

import numpy as np
import ml_dtypes
from contextlib import ExitStack

import concourse.bass as bass
import concourse.bacc as bacc
import concourse.tile as tile
from concourse import mybir
from concourse.bass_utils import run_bass_kernel_spmd

F32 = mybir.dt.float32
BF16 = mybir.dt.bfloat16
AF = mybir.ActivationFunctionType
OP = mybir.AluOpType

B = 4
L = 1024
D_MODEL = 512
D_IN = 1024
N = 16
DT_RANK = 32
K_CONV = 4

# fraction (num/den) of the 64 fused C*h multiplies routed to Pool
POOL_NUM = 53
POOL_DEN = 64


def _in_shapes():
    return {
        "xT": ((D_MODEL, L + 4), BF16),
        "w1x": ((D_MODEL, D_IN), BF16),
        "w1z": ((D_MODEL, D_IN), BF16),
        "wx": ((D_IN, 2 * N + DT_RANK), BF16),
        "wdt": ((DT_RANK, D_IN), BF16),
        "wcomb": ((D_IN, D_MODEL), BF16),
        "consts": ((D_IN, N + 3 + K_CONV), F32),
        "ident": ((128, 128), BF16),
        "ddiag": ((D_IN, 128), BF16),
    }


def _kernel_body(tc, out, ins):
    nc = tc.nc
    SEGL = L + 2
    SPI = 2
    NB = D_IN // 128
    NM = D_MODEL // 128
    TS = 512
    TH = L // TS
    NQ = N // SPI
    DEPTH = 3

    with ExitStack() as ctx:
        wpool = ctx.enter_context(tc.tile_pool(name="w", bufs=1))
        pers = ctx.enter_context(tc.tile_pool(name="pers", bufs=1))
        work = ctx.enter_context(tc.tile_pool(name="work", bufs=2))
        spool = ctx.enter_context(tc.tile_pool(name="scan", bufs=2))
        ppool = ctx.enter_context(tc.tile_pool(name="ps", bufs=2, space="PSUM"))
        ypool = ctx.enter_context(tc.tile_pool(name="yps", bufs=1, space="PSUM"))

        def load_rows(name, nchunks, width, dt=BF16, eng=None):
            src = ins[name]
            ts = []
            for c in range(nchunks):
                t = wpool.tile([128, width], dt, tag=f"{name}{c}", name=f"{name}{c}")
                (eng or nc.sync).dma_start(t[:], src[c * 128:(c + 1) * 128, :])
                ts.append(t)
            return ts

        # critical-path loads on the SP queue, in need-order
        xT_sb = load_rows("xT", NM, L + 4)
        w1x_sb = load_rows("w1x", NM, D_IN)
        cst_sb = load_rows("consts", NB, N + 3 + K_CONV, F32)
        w1z_sb = load_rows("w1z", NM, D_IN)
        wx_sb = load_rows("wx", NB, 2 * N + DT_RANK)
        wc_sb = load_rows("wcomb", NB, D_MODEL)
        A_sb = cst_sb
        cb_sb = [t[:, N:N + 1] for t in cst_sb]
        bdt_sb = [t[:, N + 1:N + 2] for t in cst_sb]
        cw_sb = [[t[:, N + 3 + k:N + 4 + k] for k in range(K_CONV)] for t in cst_sb]
        wdt_sb = wpool.tile([DT_RANK, D_IN], BF16)
        nc.sync.dma_start(wdt_sb[:], ins["wdt"][:, :])
        id_sb = wpool.tile([128, 128], BF16)
        nc.sync.dma_start(id_sb[:], ins["ident"][:, :])
        dd_sb = load_rows("ddiag", NB, 128)

        # phase B: per block: xh matmul -> xpre; conv on DVE; silu; z-gate
        # matmul + silu; x_dbl accumulation
        zs_dram = nc.dram_tensor("zs_scratch", [D_IN, L], BF16, kind="Internal").ap()
        xh_sb = [pers.tile([128, L], BF16, tag=f"xh{b}", name=f"xh{b}") for b in range(NB)]
        dt_sb = pers.tile([DT_RANK, L], BF16, tag="dtz")
        bc_sb = work.tile([2 * N, L], BF16, tag="osb")
        ppc = [ppool.tile([2 * N + DT_RANK, TS], F32, tag=f"ppc{th}", bufs=1,
                          name=f"ppc{th}") for th in range(TH)]
        for b in range(NB):
            xpre = work.tile([128, L + 3], BF16, tag="xpre")
            nc.vector.memset(xpre[:, 0:3], 0.0)
            for th in range(TH):
                ps = ppool.tile([128, TS], F32, tag="pB", bufs=4)
                for cm in range(NM):
                    nc.tensor.matmul(
                        ps[:], w1x_sb[cm][:, b * 128:(b + 1) * 128],
                        xT_sb[cm][:, 3 + th * TS: 3 + th * TS + TS],
                        start=(cm == 0), stop=(cm == NM - 1))
                nc.scalar.copy(xpre[:, 3 + th * TS: 3 + (th + 1) * TS], ps[:])
            tk = []
            for k in range(K_CONV):
                t = work.tile([128, L], BF16, tag=f"ct{k % 2}", bufs=1, name=f"ct{b}_{k}")
                nc.vector.tensor_scalar_mul(t[:], xpre[:, k:k + L], cw_sb[b][k])
                tk.append(t)
                if k % 2 == 1:
                    sm = work.tile([128, L], BF16, tag=f"cs{k // 2}", bufs=1,
                                   name=f"cs{b}_{k}")
                    nc.vector.tensor_add(sm[:], tk[k - 1][:], tk[k][:])
                    tk[k] = sm
            ca = work.tile([128, L], BF16, tag="ct0", bufs=1)
            nc.vector.tensor_add(ca[:], tk[1][:], tk[3][:])
            nc.scalar.activation(xh_sb[b][:], ca[:], AF.Silu, bias=cb_sb[b])
            # z-gate for this block (same Silu table residency)
            zt = pers.tile([128, L], BF16, tag="dtz", name=f"zt{b}")
            for th in range(TH):
                psz = ppool.tile([128, TS], F32, tag="pB", bufs=4)
                for cm in range(NM):
                    nc.tensor.matmul(
                        psz[:], w1z_sb[cm][:, b * 128:(b + 1) * 128],
                        xT_sb[cm][:, 3 + th * TS: 3 + th * TS + TS],
                        start=(cm == 0), stop=(cm == NM - 1))
                nc.scalar.activation(zt[:, th * TS:(th + 1) * TS], psz[:], AF.Silu)
            nc.sync.dma_start(zs_dram[b * 128:(b + 1) * 128, :], zt[:])
            # x_dbl partial accumulation for this block
            for th in range(TH):
                nc.tensor.matmul(ppc[th][:], wx_sb[b][:, :],
                                 xh_sb[b][:, th * TS:(th + 1) * TS],
                                 start=(b == 0), stop=(b == NB - 1))
        for th in range(TH):
            nc.scalar.copy(dt_sb[:, th * TS:(th + 1) * TS], ppc[th][0:DT_RANK, :])
            nc.scalar.copy(bc_sb[:, th * TS:(th + 1) * TS],
                           ppc[th][DT_RANK:2 * N + DT_RANK, :])

        # phase D: broadcast B,C rows across partitions (DRAM bounce, 0-stride
        # read).  B_n tiles recycle the dead xT/w1x/w1z slots; C per-q-pair
        # tiles are fresh (contiguous 2L for the fused Pool multiply).
        bc_dram = nc.dram_tensor("bc_scratch", [2 * N, L], BF16, kind="Internal").ap()
        nc.sync.dma_start(bc_dram[:, :], bc_sb[:])
        reuse_tags = [f"xT{c}" for c in range(NM)] + [f"w1x{c}" for c in range(NM)] \
            + [f"w1z{c}" for c in range(NM)] + [f"Bb{i}" for i in range(4)]
        Bn_sb = [wpool.tile([128, L], BF16, tag=reuse_tags[n], name=f"Bn{n}")
                 for n in range(N)]
        Cq_sb = [pers.tile([128, SPI * L], BF16, tag=f"Cq{q}", name=f"Cq{q}")
                 for q in range(NQ)]
        for n in range(N):
            src = bc_dram[n:n + 1, :]
            src_b = bass.AP(tensor=src.tensor, offset=src.offset,
                            ap=[[0, 128]] + [list(d) for d in src.ap[1:]])
            nc.sync.dma_start(Bn_sb[n][:], src_b)
            srcc = bc_dram[N + n:N + n + 1, :]
            srcc_b = bass.AP(tensor=srcc.tensor, offset=srcc.offset,
                             ap=[[0, 128]] + [list(d) for d in srcc.ap[1:]])
            nc.sync.dma_start(Cq_sb[n // SPI][:, (n % SPI) * L:(n % SPI + 1) * L],
                              srcc_b)

        # phase SP: softplus for all blocks, batched per activation function
        delta_sb = [pers.tile([128, L], BF16, tag=f"dl{b}", name=f"dl{b}")
                    for b in range(NB)]
        for b in range(NB):
            for th in range(TH):
                zpre = ppool.tile([128, TS], F32, tag="pB", bufs=4)
                nc.tensor.matmul(zpre[:], wdt_sb[:, b * 128:(b + 1) * 128],
                                 dt_sb[:, th * TS:(th + 1) * TS],
                                 start=True, stop=True)
                nc.scalar.activation(delta_sb[b][:, th * TS:(th + 1) * TS], zpre[:],
                                     AF.Exp, bias=bdt_sb[b])
        for b in range(NB):
            # in-place: delta = ln(exp(zpre+bdt) + 1) = softplus(zpre + bdt)
            nc.scalar.activation(delta_sb[b][:], delta_sb[b][:], AF.Ln, bias=1.0)

        # phase E: per channel-block: u, dA, scan, y
        y4_sb = [pers.tile([128, L], BF16, tag=f"y4{b}", name=f"y4{b}") for b in range(NB)]
        d0_pp = [spool.tile([128, SPI * SEGL], BF16, tag=f"d0{i}", bufs=1, name=f"d0pp{i}")
                 for i in range(DEPTH)]
        d1_pp = [spool.tile([128, SPI * SEGL], BF16, tag=f"d1{i}", bufs=1, name=f"d1pp{i}")
                 for i in range(DEPTH)]
        h_pp = [spool.tile([128, SPI * SEGL], BF16, tag=f"h{i}", bufs=1, name=f"hpp{i}")
                for i in range(DEPTH)]
        for dd in d0_pp + d1_pp:
            pad = bass.AP(tensor=dd.tensor, offset=dd.offset + L,
                          ap=[list(dd.ap[0]), [SEGL, SPI], [1, SEGL - L]])
            nc.vector.memset(pad, 0.0)

        # Bresenham split of the 64 fused C*h multiplies between Pool and DVE
        tt_state = [0]

        def p_engine():
            tt_state[0] += POOL_NUM
            if tt_state[0] >= POOL_DEN:
                tt_state[0] -= POOL_DEN
                return nc.gpsimd
            return nc.vector

        for b in range(NB):
            u = work.tile([128, L], BF16, tag="u", bufs=1)
            nc.vector.tensor_mul(u[:], delta_sb[b][:], xh_sb[b][:])

            yps = ypool.tile([128, L], F32, tag="yps")
            for q in range(NQ):
                d0 = d0_pp[q % DEPTH]
                d1 = d1_pp[q % DEPTH]
                for nn in range(SPI):
                    n = q * SPI + nn
                    nc.scalar.activation(d0[:, nn * SEGL: nn * SEGL + L], delta_sb[b][:],
                                         AF.Exp, scale=A_sb[b][:, n:n + 1])
                    # dBu = u * B_n on DVE (same queue as the scan it feeds)
                    nc.vector.tensor_mul(d1[:, nn * SEGL: nn * SEGL + L], u[:],
                                         Bn_sb[n][:])
                h = h_pp[q % DEPTH]
                nc.vector.tensor_tensor_scan(h[:], d0[:], d1[:], 0.0, OP.mult, OP.add)
                p = spool.tile([128, SPI * L], BF16, tag="p", bufs=DEPTH)
                h_in = bass.AP(tensor=h.tensor, offset=h.offset,
                               ap=[list(h.ap[0]), [SEGL, SPI], [1, L]])
                p_engine().tensor_tensor(p[:], h_in, Cq_sb[q][:], OP.mult)
                for nn in range(SPI):
                    n = q * SPI + nn
                    for th in range(TH):
                        nc.tensor.matmul(
                            yps[:, th * TS:(th + 1) * TS], id_sb[:],
                            p[:, nn * L + th * TS: nn * L + th * TS + TS],
                            start=(n == 0 and th in (0, 1)), stop=False)
            for th in range(TH):
                nc.tensor.matmul(yps[:, th * TS:(th + 1) * TS], dd_sb[b][:],
                                 xh_sb[b][:, th * TS:(th + 1) * TS],
                                 start=False, stop=True)
            zs = work.tile([128, L], BF16, tag="zs", bufs=1)
            nc.sync.dma_start(zs[:], zs_dram[b * 128:(b + 1) * 128, :])
            ysb = work.tile([128, L], BF16, tag="ysb", bufs=1)
            nc.scalar.copy(ysb[:], yps[:])
            nc.vector.tensor_mul(y4_sb[b][:], ysb[:], zs[:])

        # phase F: partial final output = y4 @ Wcomb^T (Wcomb = W_out^T Wo_half^T
        # folded on the host, eliminating the separate Wo projection)
        for jo in range(NM):
            for th in range(TH):
                ps = ppool.tile([128, TS], F32, tag="pB", bufs=4)
                for b in range(NB):
                    nc.tensor.matmul(ps[:], wc_sb[b][:, jo * 128:(jo + 1) * 128],
                                     y4_sb[b][:, th * TS:(th + 1) * TS],
                                     start=(b == 0), stop=(b == NB - 1))
                o_sb = work.tile([128, TS], F32, tag="osb")
                nc.vector.tensor_copy(o_sb[:], ps[:])
                nc.sync.dma_start(out[jo * 128:(jo + 1) * 128,
                                      th * TS:(th + 1) * TS], o_sb[:])


_NC_CACHE = None


def _build_nc():
    global _NC_CACHE
    if _NC_CACHE is not None:
        return _NC_CACHE
    nc = bacc.Bacc("TRN2", target_bir_lowering=False, debug=False, num_devices=8)
    ins = {}
    for name, (shape, dt) in _in_shapes().items():
        ins[name] = nc.dram_tensor(name, list(shape), dt, kind="ExternalInput").ap()
    out = nc.dram_tensor("out", [D_MODEL, L], F32, kind="ExternalOutput").ap()
    with tile.TileContext(nc) as tc:
        _kernel_body(tc, out, ins)
    nc.compile()
    _NC_CACHE = nc
    return nc


def _prep_core_inputs(x, p):
    """x: (L, 512) f32 input for this core; p: dict with this direction's params
    plus 'wo_half' (512, 512) = Wo[:, half].T."""
    bf = ml_dtypes.bfloat16
    xTp = np.zeros((D_MODEL, L + 4), np.float32)
    xTp[:, 3:3 + L] = x.T
    W_in = p['W_in']
    conv_w = p['conv_w'][:, 0, :]
    consts = np.concatenate([
        -np.exp(p['A_log']).astype(np.float32),
        p['conv_b'].reshape(-1, 1).astype(np.float32),
        p['b_dt'].reshape(-1, 1).astype(np.float32),
        p['D'].reshape(-1, 1).astype(np.float32),
        conv_w.astype(np.float32)], axis=1)
    # fold the two output projections: out = Wo_half @ (W_out @ y4) = Wcomb^T y4
    wcomb = (p['W_out'].astype(np.float64).T @ p['wo_half'].astype(np.float64))
    return {
        "xT": xTp.astype(bf),
        "w1x": np.ascontiguousarray(W_in[:D_IN, :].T).astype(bf),
        "w1z": np.ascontiguousarray(W_in[D_IN:, :].T).astype(bf),
        "wx": np.ascontiguousarray(p['W_x'].T).astype(bf),
        "wdt": np.ascontiguousarray(p['W_dt'].T).astype(bf),
        "wcomb": np.ascontiguousarray(wcomb).astype(np.float32).astype(bf),
        "consts": np.ascontiguousarray(consts).astype(np.float32),
        "ident": np.eye(128, dtype=bf),
        "ddiag": np.concatenate([np.diag(p['D'][b * 128:(b + 1) * 128])
                                 for b in range(D_IN // 128)], axis=0).astype(bf),
    }


def _dir_params(inputs, prefix, wo_half):
    names = ['W_in', 'conv_w', 'conv_b', 'W_x', 'W_dt', 'b_dt', 'A_log', 'D', 'W_out']
    p = {n: np.asarray(inputs[prefix + n], np.float32) for n in names}
    p['wo_half'] = wo_half
    return p


def _masked_flip(x, lengths):
    L_ = x.shape[1]
    j = np.arange(L_)[None, :]
    idx = np.where(j < lengths[:, None], lengths[:, None] - 1 - j, j)
    return np.take_along_axis(x, idx[:, :, None], axis=1)


def kernel(**inputs):
    nc = _build_nc()
    hidden = np.asarray(inputs['hidden_input'], np.float32)   # (B, L, 512)
    mask = np.asarray(inputs['mask'], np.int32)
    Wo = np.asarray(inputs['Wo'], np.float32)                 # (512, 1024)
    bo = np.asarray(inputs['bo'], np.float32)

    lengths = mask.sum(axis=1)
    bwd_in = _masked_flip(hidden, lengths)

    pf = _dir_params(inputs, 'f_', np.ascontiguousarray(Wo[:, :D_MODEL].T))
    pb = _dir_params(inputs, 'b_', np.ascontiguousarray(Wo[:, D_MODEL:].T))

    in_maps = []
    for i in range(B):
        in_maps.append(_prep_core_inputs(hidden[i], pf))
    for i in range(B):
        in_maps.append(_prep_core_inputs(bwd_in[i], pb))

    res = run_bass_kernel_spmd(nc, in_maps, core_ids=list(range(8)))

    out = np.empty((B, L, D_MODEL), np.float32)
    for i in range(B):
        fwd = res.results[i]["out"].T                       # (L, 512)
        bwd_f = res.results[B + i]["out"].T                 # (L, 512), flipped time
        bwd = _masked_flip(bwd_f[None], lengths[i:i + 1])[0]
        out[i] = fwd + bwd + bo
    return out


# revision 16
# speedup vs baseline: 1.1498x; 1.0276x over previous
"""Trainium2 Bass kernel for nn_ExBimamba: bidirectional Mamba block.

Sharding: 8 NeuronCores = 4 samples x 2 directions (fwd/bwd). Each core runs one
full Mamba pass for one (sample, direction) plus the final output projection
folded into W_out (host precomputes Wcomb = W_out^T @ Wo_half^T); the host sums
the two partial projections per sample and adds bo.

Per-core layout: channels on partitions, time on free dim.
- phase B per block: input matmul, depthwise conv (DVE tensor-scalar tree while
  PE is busy), silu, z-gate matmul + silu, x_dbl accumulation -- one Silu table
  residency for the whole phase, PE kept continuously warm
- softplus batched: all 8 blocks' Exp, then all Ln in-place (few table loads)
- dA_n = Exp(A[d,n] * delta) via ACT with per-partition scale pointer
- selective scan via the DVE tensor_tensor_scan instruction (DVE-only opcode),
  two zero-pad-separated (channel-block, n) state segments per instruction
- dBu on DVE (feeds the scan on the same queue); C*h split between the
  otherwise-idle GPSIMD/Pool engine and DVE to balance occupancy
- B/C rows broadcast across partitions with 0-partition-stride DMA (DRAM
  bounce); B_n tiles recycle the SBUF slots of xT/w1x/w1z, which are dead
  after phase B
- y = sum_n C*h via identity-stationary accumulating matmuls (PE), with the
  + xh*D skip connection folded in as a diagonal-stationary matmul
"""
import sys
import os

for _p in ('/opt/trn_rl_repo', os.path.join(os.path.dirname(os.path.abspath(__file__)))):
    if _p not in sys.path:
        sys.path.insert(0, _p)

import numpy as np
import ml_dtypes
from contextlib import ExitStack

import concourse.bass as bass
import concourse.bacc as bacc
import concourse.tile as tile
from concourse import mybir
from concourse.bass_utils import run_bass_kernel_spmd

F32 = mybir.dt.float32
BF16 = mybir.dt.bfloat16
AF = mybir.ActivationFunctionType
OP = mybir.AluOpType

B = 4
L = 1024
D_MODEL = 512
D_IN = 1024
N = 16
DT_RANK = 32
K_CONV = 4

# fraction (num/den) of the 64 fused C*h multiplies routed to Pool
POOL_NUM = 55
POOL_DEN = 64

CW = N + 3 + K_CONV   # consts columns per block


def _in_shapes():
    return {
        "xT": ((D_MODEL, L + 4), BF16),
        "w1x": ((D_MODEL, D_IN), BF16),
        "w1z": ((D_MODEL, D_IN), BF16),
        "wx": ((D_IN, 2 * N + DT_RANK), BF16),
        "wdt": ((DT_RANK, D_IN), BF16),
        "wcomb": ((D_IN, D_MODEL), BF16),
        "consts": ((128, (D_IN // 128) * CW), F32),
        "ident": ((128, 128), BF16),
        "ddiag": ((128, D_IN), BF16),
    }


def _kernel_body(tc, out, ins):
    nc = tc.nc
    SEGL = L + 2
    SPI = 2
    NB = D_IN // 128
    NM = D_MODEL // 128
    TS = 512
    TH = L // TS
    NQ = N // SPI
    DEPTH = 3

    with ExitStack() as ctx:
        wpool = ctx.enter_context(tc.tile_pool(name="w", bufs=1))
        pers = ctx.enter_context(tc.tile_pool(name="pers", bufs=1))
        work = ctx.enter_context(tc.tile_pool(name="work", bufs=2))
        spool = ctx.enter_context(tc.tile_pool(name="scan", bufs=2))
        ppool = ctx.enter_context(tc.tile_pool(name="ps", bufs=2, space="PSUM"))
        ypool = ctx.enter_context(tc.tile_pool(name="yps", bufs=1, space="PSUM"))

        def load_rows(name, nchunks, width, dt=BF16, eng=None):
            src = ins[name]
            ts = []
            for c in range(nchunks):
                t = wpool.tile([128, width], dt, tag=f"{name}{c}", name=f"{name}{c}")
                (eng or nc.sync).dma_start(t[:], src[c * 128:(c + 1) * 128, :])
                ts.append(t)
            return ts

        # critical-path loads on the SP queue, in need-order; consts/ddiag are
        # packed into single wide tiles so each is one DMA
        xT_sb = load_rows("xT", NM, L + 4)
        w1x_sb = load_rows("w1x", NM, D_IN)
        cpk = wpool.tile([128, NB * CW], F32)
        nc.sync.dma_start(cpk[:], ins["consts"][:, :])
        w1z_sb = load_rows("w1z", NM, D_IN)
        wx_sb = load_rows("wx", NB, 2 * N + DT_RANK)
        wdt_sb = wpool.tile([DT_RANK, D_IN], BF16)
        nc.sync.dma_start(wdt_sb[:], ins["wdt"][:, :])
        cst_sb = [cpk[:, b * CW:(b + 1) * CW] for b in range(NB)]
        A_sb = cst_sb
        cb_sb = [t[:, N:N + 1] for t in cst_sb]
        bdt_sb = [t[:, N + 1:N + 2] for t in cst_sb]
        cw_sb = [[t[:, N + 3 + k:N + 4 + k] for k in range(K_CONV)] for t in cst_sb]
        # needed only from phase E / F on; keep off the critical load path
        id_sb = wpool.tile([128, 128], BF16)
        nc.scalar.dma_start(id_sb[:], ins["ident"][:, :])
        ddpk = wpool.tile([128, D_IN], BF16)
        nc.scalar.dma_start(ddpk[:], ins["ddiag"][:, :])
        dd_sb = [ddpk[:, b * 128:(b + 1) * 128] for b in range(NB)]
        wc_sb = load_rows("wcomb", NB, D_MODEL, eng=nc.scalar)

        # phase B: per block: xh matmul -> xpre; conv on DVE; silu; z-gate
        # matmul + silu; x_dbl accumulation
        zs_dram = nc.dram_tensor("zs_scratch", [D_IN, L], BF16, kind="Internal").ap()
        xh_sb = [pers.tile([128, L], BF16, tag=f"xh{b}", name=f"xh{b}") for b in range(NB)]
        dt_sb = pers.tile([DT_RANK, L], BF16, tag="dtz")
        bc_sb = work.tile([2 * N, L], BF16, tag="osb")
        ppc = [ppool.tile([2 * N + DT_RANK, TS], F32, tag=f"ppc{th}", bufs=1,
                          name=f"ppc{th}") for th in range(TH)]
        for b in range(NB):
            xpre = work.tile([128, L + 3], BF16, tag="xpre")
            nc.vector.memset(xpre[:, 0:3], 0.0)
            for th in range(TH):
                ps = ppool.tile([128, TS], F32, tag="pB", bufs=2)
                for cm in range(NM):
                    nc.tensor.matmul(
                        ps[:], w1x_sb[cm][:, b * 128:(b + 1) * 128],
                        xT_sb[cm][:, 3 + th * TS: 3 + th * TS + TS],
                        start=(cm == 0), stop=(cm == NM - 1))
                nc.scalar.copy(xpre[:, 3 + th * TS: 3 + (th + 1) * TS], ps[:])
            tk = []
            for k in range(K_CONV):
                t = work.tile([128, L], BF16, tag=f"ct{k % 2}", bufs=1, name=f"ct{b}_{k}")
                nc.vector.tensor_scalar_mul(t[:], xpre[:, k:k + L], cw_sb[b][k])
                tk.append(t)
                if k % 2 == 1:
                    sm = work.tile([128, L], BF16, tag=f"cs{k // 2}", bufs=1,
                                   name=f"cs{b}_{k}")
                    nc.vector.tensor_add(sm[:], tk[k - 1][:], tk[k][:])
                    tk[k] = sm
            ca = work.tile([128, L], BF16, tag="ca")
            nc.vector.tensor_add(ca[:], tk[1][:], tk[3][:])
            nc.scalar.activation(xh_sb[b][:], ca[:], AF.Silu, bias=cb_sb[b])
            # z-gate for this block (same Silu table residency)
            zt = pers.tile([128, L], BF16, tag="dtz", name=f"zt{b}")
            for th in range(TH):
                psz = ppool.tile([128, TS], F32, tag="pB", bufs=2)
                for cm in range(NM):
                    nc.tensor.matmul(
                        psz[:], w1z_sb[cm][:, b * 128:(b + 1) * 128],
                        xT_sb[cm][:, 3 + th * TS: 3 + th * TS + TS],
                        start=(cm == 0), stop=(cm == NM - 1))
                nc.scalar.activation(zt[:, th * TS:(th + 1) * TS], psz[:], AF.Silu)
            nc.sync.dma_start(zs_dram[b * 128:(b + 1) * 128, :], zt[:])
            # x_dbl partial accumulation for this block
            for th in range(TH):
                nc.tensor.matmul(ppc[th][:], wx_sb[b][:, :],
                                 xh_sb[b][:, th * TS:(th + 1) * TS],
                                 start=(b == 0), stop=(b == NB - 1))
        for th in range(TH):
            nc.scalar.copy(dt_sb[:, th * TS:(th + 1) * TS], ppc[th][0:DT_RANK, :])
            nc.scalar.copy(bc_sb[:, th * TS:(th + 1) * TS],
                           ppc[th][DT_RANK:2 * N + DT_RANK, :])

        # phase D: broadcast B,C rows across partitions (DRAM bounce, 0-stride
        # read).  B_n tiles recycle the dead xT/w1x/w1z slots; C per-q-pair
        # tiles are fresh (contiguous 2L for the fused Pool multiply).
        bc_dram = nc.dram_tensor("bc_scratch", [2 * N, L], BF16, kind="Internal").ap()
        nc.sync.dma_start(bc_dram[:, :], bc_sb[:])
        reuse_tags = [f"xT{c}" for c in range(NM)] + [f"w1x{c}" for c in range(NM)] \
            + [f"w1z{c}" for c in range(NM)] + [f"Bb{i}" for i in range(4)]
        Bn_sb = [wpool.tile([128, L], BF16, tag=reuse_tags[n], name=f"Bn{n}")
                 for n in range(N)]
        Cq_sb = [pers.tile([128, SPI * L], BF16, tag=f"Cq{q}", name=f"Cq{q}")
                 for q in range(NQ)]
        for n in range(N):
            src = bc_dram[n:n + 1, :]
            src_b = bass.AP(tensor=src.tensor, offset=src.offset,
                            ap=[[0, 128]] + [list(d) for d in src.ap[1:]])
            nc.sync.dma_start(Bn_sb[n][:], src_b)
            srcc = bc_dram[N + n:N + n + 1, :]
            srcc_b = bass.AP(tensor=srcc.tensor, offset=srcc.offset,
                             ap=[[0, 128]] + [list(d) for d in srcc.ap[1:]])
            nc.sync.dma_start(Cq_sb[n // SPI][:, (n % SPI) * L:(n % SPI + 1) * L],
                              srcc_b)

        # phase SP: softplus for all blocks, batched per activation function
        delta_sb = [pers.tile([128, L], BF16, tag=f"dl{b}", name=f"dl{b}")
                    for b in range(NB)]
        for b in range(NB):
            for th in range(TH):
                zpre = ppool.tile([128, TS], F32, tag="pB", bufs=2)
                nc.tensor.matmul(zpre[:], wdt_sb[:, b * 128:(b + 1) * 128],
                                 dt_sb[:, th * TS:(th + 1) * TS],
                                 start=True, stop=True)
                nc.scalar.activation(delta_sb[b][:, th * TS:(th + 1) * TS], zpre[:],
                                     AF.Exp, bias=bdt_sb[b])
        for b in range(NB):
            # in-place: delta = ln(exp(zpre+bdt) + 1) = softplus(zpre + bdt)
            nc.scalar.activation(delta_sb[b][:], delta_sb[b][:], AF.Ln, bias=1.0)

        # phase E: per channel-block: u, dA, scan, y
        y4_sb = [pers.tile([128, L], BF16, tag=f"y4{b}", name=f"y4{b}") for b in range(NB)]
        d0_pp = [spool.tile([128, SPI * SEGL], BF16, tag=f"d0{i}", bufs=1, name=f"d0pp{i}")
                 for i in range(DEPTH)]
        d1_pp = [spool.tile([128, SPI * SEGL], BF16, tag=f"d1{i}", bufs=1, name=f"d1pp{i}")
                 for i in range(DEPTH)]
        h_pp = [spool.tile([128, SPI * SEGL], BF16, tag=f"h{i}", bufs=1, name=f"hpp{i}")
                for i in range(DEPTH)]
        for dd in d0_pp + d1_pp:
            pad = bass.AP(tensor=dd.tensor, offset=dd.offset + L,
                          ap=[list(dd.ap[0]), [SEGL, SPI], [1, SEGL - L]])
            nc.vector.memset(pad, 0.0)

        # Bresenham split of the 64 fused C*h multiplies between Pool and DVE
        tt_state = [0]

        def p_engine():
            tt_state[0] += POOL_NUM
            if tt_state[0] >= POOL_DEN:
                tt_state[0] -= POOL_DEN
                return nc.gpsimd
            return nc.vector

        for b in range(NB):
            u = work.tile([128, L], BF16, tag="u", bufs=1)
            nc.vector.tensor_mul(u[:], delta_sb[b][:], xh_sb[b][:])

            yps = ypool.tile([128, L], F32, tag="yps", bufs=2)
            for q in range(NQ):
                d0 = d0_pp[q % DEPTH]
                d1 = d1_pp[q % DEPTH]
                for nn in range(SPI):
                    n = q * SPI + nn
                    nc.scalar.activation(d0[:, nn * SEGL: nn * SEGL + L], delta_sb[b][:],
                                         AF.Exp, scale=A_sb[b][:, n:n + 1])
                    # dBu = u * B_n on DVE (same queue as the scan it feeds)
                    nc.vector.tensor_mul(d1[:, nn * SEGL: nn * SEGL + L], u[:],
                                         Bn_sb[n][:])
                h = h_pp[q % DEPTH]
                nc.vector.tensor_tensor_scan(h[:], d0[:], d1[:], 0.0, OP.mult, OP.add)
                p = spool.tile([128, SPI * L], BF16, tag="p", bufs=DEPTH)
                h_in = bass.AP(tensor=h.tensor, offset=h.offset,
                               ap=[list(h.ap[0]), [SEGL, SPI], [1, L]])
                p_engine().tensor_tensor(p[:], h_in, Cq_sb[q][:], OP.mult)
                for nn in range(SPI):
                    n = q * SPI + nn
                    for th in range(TH):
                        nc.tensor.matmul(
                            yps[:, th * TS:(th + 1) * TS], id_sb[:],
                            p[:, nn * L + th * TS: nn * L + th * TS + TS],
                            start=(n == 0 and th in (0, 1)), stop=False)
            for th in range(TH):
                nc.tensor.matmul(yps[:, th * TS:(th + 1) * TS], dd_sb[b][:],
                                 xh_sb[b][:, th * TS:(th + 1) * TS],
                                 start=False, stop=True)
            zs = work.tile([128, L], BF16, tag="zs", bufs=1)
            nc.sync.dma_start(zs[:], zs_dram[b * 128:(b + 1) * 128, :])
            ysb = work.tile([128, L], BF16, tag="ysb", bufs=1)
            nc.scalar.copy(ysb[:], yps[:])
            nc.vector.tensor_mul(y4_sb[b][:], ysb[:], zs[:])

        # phase F: partial final output = y4 @ Wcomb^T (Wcomb = W_out^T Wo_half^T
        # folded on the host, eliminating the separate Wo projection)
        for jo in range(NM):
            for th in range(TH):
                ps = ppool.tile([128, TS], F32, tag="pB", bufs=2)
                for b in range(NB):
                    nc.tensor.matmul(ps[:], wc_sb[b][:, jo * 128:(jo + 1) * 128],
                                     y4_sb[b][:, th * TS:(th + 1) * TS],
                                     start=(b == 0), stop=(b == NB - 1))
                o_sb = work.tile([128, TS], F32, tag="osb")
                nc.vector.tensor_copy(o_sb[:], ps[:])
                nc.sync.dma_start(out[jo * 128:(jo + 1) * 128,
                                      th * TS:(th + 1) * TS], o_sb[:])


_NC_CACHE = None


def _build_nc():
    global _NC_CACHE
    if _NC_CACHE is not None:
        return _NC_CACHE
    nc = bacc.Bacc("TRN2", target_bir_lowering=False, debug=False, num_devices=8)
    ins = {}
    for name, (shape, dt) in _in_shapes().items():
        ins[name] = nc.dram_tensor(name, list(shape), dt, kind="ExternalInput").ap()
    out = nc.dram_tensor("out", [D_MODEL, L], F32, kind="ExternalOutput").ap()
    with tile.TileContext(nc) as tc:
        _kernel_body(tc, out, ins)
    nc.compile()
    _NC_CACHE = nc
    return nc


def _prep_core_inputs(x, p):
    """x: (L, 512) f32 input for this core; p: dict with this direction's params
    plus 'wo_half' (512, 512) = Wo[:, half].T."""
    bf = ml_dtypes.bfloat16
    xTp = np.zeros((D_MODEL, L + 4), np.float32)
    xTp[:, 3:3 + L] = x.T
    W_in = p['W_in']
    conv_w = p['conv_w'][:, 0, :]
    consts = np.concatenate([
        -np.exp(p['A_log']).astype(np.float32),
        p['conv_b'].reshape(-1, 1).astype(np.float32),
        p['b_dt'].reshape(-1, 1).astype(np.float32),
        p['D'].reshape(-1, 1).astype(np.float32),
        conv_w.astype(np.float32)], axis=1)
    # pack per-block consts side by side: [128, NB*CW]
    consts_pk = np.concatenate([consts[b * 128:(b + 1) * 128, :]
                                for b in range(D_IN // 128)], axis=1)
    ddiag_pk = np.concatenate([np.diag(p['D'][b * 128:(b + 1) * 128])
                               for b in range(D_IN // 128)], axis=1)
    # fold the two output projections: out = Wo_half @ (W_out @ y4) = Wcomb^T y4
    wcomb = (p['W_out'].astype(np.float64).T @ p['wo_half'].astype(np.float64))
    return {
        "xT": xTp.astype(bf),
        "w1x": np.ascontiguousarray(W_in[:D_IN, :].T).astype(bf),
        "w1z": np.ascontiguousarray(W_in[D_IN:, :].T).astype(bf),
        "wx": np.ascontiguousarray(p['W_x'].T).astype(bf),
        "wdt": np.ascontiguousarray(p['W_dt'].T).astype(bf),
        "wcomb": np.ascontiguousarray(wcomb).astype(np.float32).astype(bf),
        "consts": np.ascontiguousarray(consts_pk).astype(np.float32),
        "ident": np.eye(128, dtype=bf),
        "ddiag": np.ascontiguousarray(ddiag_pk).astype(bf),
    }


def _dir_params(inputs, prefix, wo_half):
    names = ['W_in', 'conv_w', 'conv_b', 'W_x', 'W_dt', 'b_dt', 'A_log', 'D', 'W_out']
    p = {n: np.asarray(inputs[prefix + n], np.float32) for n in names}
    p['wo_half'] = wo_half
    return p


def _masked_flip(x, lengths):
    L_ = x.shape[1]
    j = np.arange(L_)[None, :]
    idx = np.where(j < lengths[:, None], lengths[:, None] - 1 - j, j)
    return np.take_along_axis(x, idx[:, :, None], axis=1)


def kernel(**inputs):
    nc = _build_nc()
    hidden = np.asarray(inputs['hidden_input'], np.float32)   # (B, L, 512)
    mask = np.asarray(inputs['mask'], np.int32)
    Wo = np.asarray(inputs['Wo'], np.float32)                 # (512, 1024)
    bo = np.asarray(inputs['bo'], np.float32)

    lengths = mask.sum(axis=1)
    bwd_in = _masked_flip(hidden, lengths)

    pf = _dir_params(inputs, 'f_', np.ascontiguousarray(Wo[:, :D_MODEL].T))
    pb = _dir_params(inputs, 'b_', np.ascontiguousarray(Wo[:, D_MODEL:].T))

    in_maps = []
    for i in range(B):
        in_maps.append(_prep_core_inputs(hidden[i], pf))
    for i in range(B):
        in_maps.append(_prep_core_inputs(bwd_in[i], pb))

    res = run_bass_kernel_spmd(nc, in_maps, core_ids=list(range(8)))

    out = np.empty((B, L, D_MODEL), np.float32)
    for i in range(B):
        fwd = res.results[i]["out"].T                       # (L, 512)
        bwd_f = res.results[B + i]["out"].T                 # (L, 512), flipped time
        bwd = _masked_flip(bwd_f[None], lengths[i:i + 1])[0]
        out[i] = fwd + bwd + bo
    return out
